# revision 13
# baseline (speedup 1.0000x reference)
"""Trainium2 Bass kernel for DecoderAttention (B=16, T=1024, D=1024, H=16).

Sharding: pure data-parallel over batch — 16 batch items / 8 cores = 2 per core.
No collectives. Each core runs the identical program on its 2 batch items.

Per-core dataflow (per batch item):
  1. hs [T,D] loaded, PE-transposed to hsT [D,T] (c on partitions).
  2. QKV projection:  QT/KT [j,t] (transposed layout, j on partitions) and
     V [t,j] (natural layout, + a fused ones-column per head for the softmax
     denominator).  All matmuls in float32r (full PE rate at N>=256); every
     f32r operand is produced by a rounding-capable engine (DVE/ACT) or is an
     ExternalInput declared f32r, which the BIR verifier accepts.
  3. Attention per head: logitsT [k,q] = KT_tile^T @ QT;  exp on ScalarE with
     the 1/sqrt(64) scale folded into the activation's free affine;
     AV: outT' [65,q] = V_aug^T @ expT accumulated over k-tiles — row 64 is
     the softmax denominator.  reciprocal (DVE) + PE K=1 broadcast matmul,
     then one DVE multiply writes the normalized attn_outT [d,t] slice.
  4. Out-projection from attn_outT (already the lhsT layout it needs) + bias.
"""

import os
import sys

import numpy as np

sys.path.insert(0, "/opt/trn_rl_repo")

import concourse.bass as bass  # noqa: E402
import concourse.mybir as mybir  # noqa: E402
import concourse.tile as tile  # noqa: E402
from concourse import bacc  # noqa: E402
from concourse.bass_utils import run_bass_kernel_spmd  # noqa: E402
from concourse.masks import make_identity  # noqa: E402

F32 = mybir.dt.float32
F32R = mybir.dt.float32r

B, T, D = 16, 1024, 1024
H, HD = 16, 64
N_CORES = 8
BL = B // N_CORES  # batch items per core
P = 128
CT = D // P  # contraction tiles (8)
TT = T // P  # token tiles (8)
NQ = 512  # matmul moving free dim
SCALE = 1.0 / np.sqrt(HD)

_last_results = None  # test.py reads this for the profile


def build_program():
    nc = bacc.Bacc(
        "TRN2", target_bir_lowering=False, debug=False, num_devices=N_CORES
    )

    hs = nc.dram_tensor("hidden_states", [BL, T, D], F32, kind="ExternalInput")
    w_qkv = nc.dram_tensor("w_qkv", [D, 3 * D], F32R, kind="ExternalInput")
    b_qkv = nc.dram_tensor("b_qkv", [3 * D], F32, kind="ExternalInput")
    w_out = nc.dram_tensor("w_out", [D, D], F32R, kind="ExternalInput")
    b_out = nc.dram_tensor("b_out", [D], F32, kind="ExternalInput")
    out = nc.dram_tensor("out", [BL, T, D], F32, kind="ExternalOutput")

    Exp = mybir.ActivationFunctionType.Exp
    add = mybir.AluOpType.add
    mult = mybir.AluOpType.mult

    with tile.TileContext(nc) as tc:
        with (
            tc.tile_pool(name="consts", bufs=1) as consts,
            tc.tile_pool(name="main", bufs=1) as main,
            tc.tile_pool(name="pipe", bufs=2) as pipe,
            tc.tile_pool(name="psum", bufs=1, space="PSUM") as psum,
        ):
            # ---------------- constants ----------------
            identity = consts.tile([P, P], F32)
            make_identity(nc, identity)
            ones_row = consts.tile([1, P], F32)
            nc.gpsimd.memset(ones_row, 1.0)
            ones_r = consts.tile([1, P], F32R)
            nc.vector.tensor_copy(ones_r, ones_row)  # DVE rounds f32 -> f32r
            ones_ph = consts.tile([P, H, 1], F32)
            nc.gpsimd.memset(ones_ph, 1.0)
            # per-partition bias for QT/KT tiles: bq[p, jt] = b_qkv[jt*128+p]
            bq = consts.tile([P, 2 * CT], F32)  # [128, 16] -> j 0..2047
            nc.sync.dma_start(
                out=bq, in_=b_qkv.rearrange("(i p) -> p i", p=P)[:, 0 : 2 * CT]
            )
            # broadcast b_qkv V-slice and b_out along partitions via K=1 matmul
            bcast_bv = consts.tile([P, D], F32)
            bcast_bout = consts.tile([P, D], F32)
            bv_row = pipe.tile([1, D], F32, tag="exp", name="bv_row")
            nc.sync.dma_start(out=bv_row, in_=b_qkv[2 * D : 3 * D][None, :])
            bout_row = pipe.tile([1, D], F32, tag="exp", name="bout_row")
            nc.sync.dma_start(out=bout_row, in_=b_out[None, :])
            for dst, src in ((bcast_bv, bv_row), (bcast_bout, bout_row)):
                ps_b = psum.tile([P, D], F32, tag="p_big", bufs=2)
                for c in range(2):
                    sl = slice(c * NQ, (c + 1) * NQ)
                    nc.tensor.matmul(
                        ps_b[:, sl], ones_row, src[:, sl],
                        start=True, stop=True,
                    )
                nc.vector.tensor_copy(dst, ps_b)

            for b in range(BL):
                # ---------------- phase A: hs -> hsT ----------------
                hs_sb = []
                for t in range(TT):
                    h_t = main.tile([P, D], F32, tag=f"qt{t}", name=f"hs{b}_{t}")
                    nc.sync.dma_start(out=h_t, in_=hs[b, t * P : (t + 1) * P, :])
                    hs_sb.append(h_t)
                hsT = []
                for c in range(CT):
                    ps_tr = psum.tile([P, T], F32, tag="p_big", bufs=2,
                                      name=f"ps_tr{b}_{c}")
                    for t in range(TT):
                        nc.tensor.transpose(
                            ps_tr[:, t * P : (t + 1) * P],
                            hs_sb[t][:, c * P : (c + 1) * P],
                            identity,
                        )
                    hT = main.tile([P, T], F32R, tag=f"hsT{c}", name=f"hsT{b}_{c}")
                    nc.vector.tensor_copy(hT, ps_tr)
                    hsT.append(hT)

                # ---------------- phase B: projections ----------------
                # QT/KT: out[j-tile, t] = w_qkv[c,j-tile]^T @ hsT[c, t]
                QT, KT = [], []
                for jt in range(2 * CT):  # 16 j-tiles (Q: 0-7, K: 8-15)
                    wq_t = pipe.tile([P, CT, P], F32R, tag="wq",
                                     name=f"wq{b}_{jt}")
                    nc.sync.dma_start(
                        out=wq_t,
                        in_=w_qkv.rearrange("(c p) j -> p c j", p=P)[
                            :, :, jt * P : (jt + 1) * P
                        ],
                    )
                    ps_qk = psum.tile([P, T], F32, tag="p_big", bufs=2,
                                      name=f"ps_qk{b}_{jt}")
                    for c in range(CT):
                        for q in range(2):
                            sl = slice(q * NQ, (q + 1) * NQ)
                            nc.tensor.matmul(
                                ps_qk[:, sl], wq_t[:, c, :], hsT[c][:, sl],
                                start=(c == 0), stop=(c == CT - 1),
                            )
                    if jt < CT:
                        dst = main.tile([P, T], F32R, tag=f"qt{jt}",
                                        name=f"QT{b}_{jt}")
                        QT.append(dst)
                    else:
                        dst = main.tile([P, T], F32R, tag=f"kt{jt - CT}",
                                        name=f"KT{b}_{jt - CT}")
                        KT.append(dst)
                    nc.vector.tensor_scalar_add(dst, ps_qk, bq[:, jt : jt + 1])

                # V natural [t, j] with fused ones column per head: [128, 16*65]
                wv_sb = []
                for c in range(CT):
                    wv_t = main.tile([P, D], F32R, tag=f"wv{c}", name=f"wv{b}_{c}")
                    nc.sync.dma_start(
                        out=wv_t, in_=w_qkv[c * P : (c + 1) * P, 2 * D : 3 * D]
                    )
                    wv_sb.append(wv_t)
                V = []
                for t in range(TT):
                    ps_v = psum.tile([P, D], F32, tag="p_av", bufs=1,
                                     name=f"ps_v{b}_{t}")
                    for c in range(CT):
                        for q in range(2):
                            sl = slice(q * NQ, (q + 1) * NQ)
                            nc.tensor.matmul(
                                ps_v[:, sl],
                                hsT[c][:, t * P : (t + 1) * P],
                                wv_sb[c][:, sl],
                                start=(c == 0), stop=(c == CT - 1),
                            )
                    v_t = main.tile([P, H * (HD + 1)], F32R, tag=f"v{t}",
                                    name=f"V{b}_{t}")
                    v3 = v_t.rearrange("p (h e) -> p h e", h=H)
                    nc.vector.tensor_copy(v3[:, :, HD : HD + 1], ones_ph)
                    nc.vector.tensor_tensor(
                        out=v3[:, :, 0:HD],
                        in0=ps_v.rearrange("p (h e) -> p h e", h=H),
                        in1=bcast_bv.rearrange("p (h e) -> p h e", h=H),
                        op=add,
                    )
                    V.append(v_t)

                # ---------------- phase C: attention ----------------
                attnT = []
                for g in range(CT):  # reuse wv slots for attn_outT
                    a_t = main.tile([P, T], F32R, tag=f"wv{g}", name=f"attnT{b}_{g}")
                    attnT.append(a_t)
                for h in range(H):
                    g, r0 = h // 2, (h % 2) * HD
                    ps_av = psum.tile([HD + 1, T], F32, tag="p_av", bufs=1,
                                      name=f"ps_av{b}_{h}")
                    for kt in range(TT):
                        ps_l = psum.tile([P, T], F32, tag="p_big", bufs=2,
                                         name=f"ps_l{b}_{h}_{kt}")
                        for q in range(2):
                            sl = slice(q * NQ, (q + 1) * NQ)
                            nc.tensor.matmul(
                                ps_l[:, sl],
                                KT[g][r0 : r0 + HD, kt * P : (kt + 1) * P],
                                QT[g][r0 : r0 + HD, sl],
                                start=True, stop=True,
                            )
                        expt = pipe.tile([P, T], F32R, tag="exp",
                                         name=f"exp{b}_{h}_{kt}")
                        nc.scalar.activation(expt, ps_l, Exp, scale=float(SCALE))
                        for q in range(2):
                            sl = slice(q * NQ, (q + 1) * NQ)
                            nc.tensor.matmul(
                                ps_av[:, sl],
                                V[kt][:, h * (HD + 1) : (h + 1) * (HD + 1)],
                                expt[:, sl],
                                start=(kt == 0), stop=(kt == TT - 1),
                            )
                    recip = pipe.tile([1, T], F32R, tag="recip", bufs=1,
                                      name=f"recip{b}_{h}")
                    with nc.allow_low_precision(reason="softmax denom recip in f32r"):
                        nc.vector.reciprocal(recip, ps_av[HD : HD + 1, :])
                    ps_bc = psum.tile([HD, T], F32, tag="p_bc", bufs=1,
                                      name=f"ps_bc{b}_{h}")
                    for q in range(2):
                        sl = slice(q * NQ, (q + 1) * NQ)
                        nc.tensor.matmul(
                            ps_bc[:, sl], ones_r[:, 0:HD], recip[:, sl],
                            start=True, stop=True,
                        )
                    bc_sb = pipe.tile([HD, T], F32, tag="bc_sb", bufs=1,
                                      name=f"bc{b}_{h}")
                    nc.scalar.copy(bc_sb, ps_bc)
                    nc.vector.tensor_tensor(
                        out=attnT[g][r0 : r0 + HD, :],
                        in0=ps_av[0:HD, :], in1=bc_sb, op=mult,
                    )

                # ---------------- phase D: out projection ----------------
                wout_sb = []
                for dt in range(CT):
                    wo_t = main.tile([P, D], F32R, tag=f"hsT{dt}", name=f"wo{b}_{dt}")
                    nc.sync.dma_start(out=wo_t, in_=w_out[dt * P : (dt + 1) * P, :])
                    wout_sb.append(wo_t)
                for t in range(TT):
                    ps_o = psum.tile([P, D], F32, tag="p_big", bufs=2,
                                     name=f"ps_o{b}_{t}")
                    for dt in range(CT):
                        for e in range(2):
                            sl = slice(e * NQ, (e + 1) * NQ)
                            nc.tensor.matmul(
                                ps_o[:, sl],
                                attnT[dt][:, t * P : (t + 1) * P],
                                wout_sb[dt][:, sl],
                                start=(dt == 0), stop=(dt == CT - 1),
                            )
                    o_t = pipe.tile([P, D], F32, tag="out_sb", name=f"o{b}_{t}")
                    nc.vector.tensor_tensor(out=o_t, in0=ps_o, in1=bcast_bout, op=add)
                    nc.sync.dma_start(out=out[b, t * P : (t + 1) * P, :], in_=o_t)

    nc.compile()
    return nc


_nc_cache = None


def kernel(**inputs) -> np.ndarray:
    global _nc_cache, _last_results
    hs = np.ascontiguousarray(np.asarray(inputs["hidden_states"], dtype=np.float32))
    w_qkv = np.ascontiguousarray(np.asarray(inputs["w_qkv"], dtype=np.float32))
    b_qkv = np.ascontiguousarray(np.asarray(inputs["b_qkv"], dtype=np.float32))
    w_out = np.ascontiguousarray(np.asarray(inputs["w_out"], dtype=np.float32))
    b_out = np.ascontiguousarray(np.asarray(inputs["b_out"], dtype=np.float32))

    if _nc_cache is None:
        _nc_cache = build_program()
    nc = _nc_cache

    in_maps = [
        {
            "hidden_states": hs[c * BL : (c + 1) * BL],
            "w_qkv": w_qkv,
            "b_qkv": b_qkv,
            "w_out": w_out,
            "b_out": b_out,
        }
        for c in range(N_CORES)
    ]
    res = run_bass_kernel_spmd(
        nc,
        in_maps,
        list(range(N_CORES)),
        trace=bool(os.environ.get("BASS_TRACE")),
    )
    _last_results = res
    return np.concatenate([res.results[c]["out"] for c in range(N_CORES)], axis=0)
